# revision 12
# baseline (speedup 1.0000x reference)
"""Causal self-attention with sink, sharded over 8 TRN2 NeuronCores.

Sharding: batch x head-group. Core c handles batch b=c//4 and heads
[4*(c%4), 4*(c%4)+4). Each core computes its QKV projection slice,
attention for its 4 heads, and a partial output projection; the host sums
the 4 partials per batch.

Device layout (per core), everything "transposed" (T on the free dim):
  - xT   [C=1024, T=2048]   (host pre-transposed x[b], bf16)
  - qT/kT in SBUF as head-pair tiles [128, T] (2 heads x 64 stacked)
  - v1   [128, 16, 4, 65]   v in natural [t, d] layout per tk-chunk/head
                            plus a ones column (65th) that accumulates the
                            softmax denominator inside the PV matmul
  - S^T = K^T Q per (head, tq-block, tk-chunk) -> exp -> E^T (no max
    subtraction: logits are O(1) for this problem's scale)
  - PV:  out^T[d, tq] (+ denom row) accumulated in PSUM over tk-chunks
  - exp(sink) joins the denominator via a vector add just before the
    (fast approx) reciprocal; gpsimd broadcast + multiply normalizes
  - out projection produces natural [t, co] partials via yT-as-stationary,
    software-pipelined one tq-block behind the attention loop
All matmul operands are bfloat16; PSUM accumulation stays fp32. Output is
DMA'd as bf16; the host sums partials in fp32.
"""

import os
import sys

import numpy as np

B, T, C = 2, 2048, 1024
H, D = 16, 64
NCORES = 8
HLOC = 4           # heads per core
GQ = HLOC * D      # 256 per-core q (or k or v) features
F = 3 * GQ         # 768 per-core qkv features
NCC = C // 128     # 8 contraction chunks
NTQ = T // 512     # 4 query blocks
NTK = T // 128     # 16 key chunks
SCALE = 1.0 / np.sqrt(D)

_BASS_PATHS = ("/opt/trn_rl_repo", "/root/.axon_site/_ro/trn_rl_repo")


def _import_bass():
    for p in _BASS_PATHS:
        if os.path.isdir(p) and p not in sys.path:
            sys.path.insert(0, p)
    import concourse.bass as bass
    import concourse.mybir as mybir
    import concourse.tile as tile
    from concourse import bacc
    return bass, mybir, tile, bacc


def build_nc(mm_dt="bfloat16", with_bias_qkv=True, with_bias_proj=True):
    """Build the per-core Bass program (same program for all 8 cores)."""
    bass, mybir, tile, bacc = _import_bass()
    f32 = mybir.dt.float32
    mdt = getattr(mybir.dt, mm_dt)
    AF = mybir.ActivationFunctionType

    nc = bacc.Bacc("TRN2", target_bir_lowering=False, debug=False)

    # host-prearranged so every input DMA is a contiguous dram read:
    # xh[128j+p, NCC*512] = x chunk layout per tq-slice j; wqh groups the
    # q01/k01/q23/k23 weight blocks; wvh/wph likewise SBUF-image layouts
    xh = nc.dram_tensor("xh", [NTQ * 128, NCC * 512], mdt, kind="ExternalInput")
    wqh = nc.dram_tensor("wqh", [128, 4 * NCC * 128], mdt, kind="ExternalInput")
    wvh = nc.dram_tensor("wvh", [128, NCC * 256], mdt, kind="ExternalInput")
    bqkv = nc.dram_tensor("bqkv", [1, F], mdt, kind="ExternalInput")
    wph = nc.dram_tensor("wph", [128, (GQ // 128) * C], mdt, kind="ExternalInput")
    bp = nc.dram_tensor("bp", [1, C], mdt, kind="ExternalInput")
    sink4 = nc.dram_tensor("sink4", [1, HLOC], f32, kind="ExternalInput")
    out = nc.dram_tensor("out", [T, C], mdt, kind="ExternalOutput")

    with tile.TileContext(nc) as tc:
        with (
            tc.tile_pool(name="const", bufs=1) as const,
            tc.tile_pool(name="persist", bufs=1) as persist,
        ):
            # ---- constants ----------------------------------------------
            ones_f = const.tile([1, 512], f32, tag="ones_f")
            nc.vector.memset(ones_f, 1.0)
            ones_r = const.tile([1, 512], mdt, tag="ones")
            nc.vector.tensor_copy(out=ones_r[:], in_=ones_f[:])
            ones_col = const.tile([128, 1], f32, tag="ones_col")
            nc.vector.memset(ones_col, 1.0)
            bqkv_r = const.tile([1, F], mdt, tag="bqkv")
            nc.sync.dma_start(out=bqkv_r[:], in_=bqkv[:, :])
            bp_r = const.tile([1, C], mdt, tag="bp")
            nc.sync.dma_start(out=bp_r[:], in_=bp[:, :])
            sink_r = const.tile([1, HLOC], f32, tag="sink")
            nc.sync.dma_start(out=sink_r[:], in_=sink4[:, :])
            # additive causal mask for the 128x128 diagonal blocks of S^T
            # (tk on partitions, tq on free): keep where tq >= tk.
            tri = const.tile([128, 128], f32, tag="tri")
            nc.gpsimd.memset(tri, 0.0)
            nc.gpsimd.affine_select(
                out=tri, in_=tri,
                compare_op=mybir.AluOpType.is_ge,
                fill=-1e30,
                base=0,
                pattern=[[1, 128]],
                channel_multiplier=-1,
            )

            # ---- persistent activations ---------------------------------
            # qk feature-block tiles: [q01, q23, k01, k23] each [128, T]
            qk = [persist.tile([128, T], mdt, tag=f"qk{i}", name=f"qk{i}") for i in range(4)]
            # v natural + ones column
            v1 = persist.tile([128, NTK, HLOC, 65], mdt, tag="v1", name="v1")
            nc.vector.tensor_copy(
                out=v1[:, :, :, 64:65],
                in_=ones_col[:, :].to_broadcast([128, NTK, HLOC, 1]),
            )
            # normalized attention output, head pairs stacked: yT[hp] [128, T]
            yT = [persist.tile([128, T], mdt, tag=f"yT{i}", name=f"yT{i}") for i in range(2)]

            # ---- input DMA (weights first, then x column-slices) --------
            wqall = persist.tile([128, 4, NCC, 128], mdt, tag="wqall", name="wqall")
            wvall = persist.tile([128, NCC, 256], mdt, tag="wvall", name="wvall")
            xtall = persist.tile([128, NCC, T], mdt, tag="xtall", name="xtall")
            xt = [xtall[:, i] for i in range(NCC)]
            # DMA issue order tracks first use (q01 weights + first x block
            # first); all reads are contiguous in dram
            def dma_wq_grp(g):
                nc.sync.dma_start(
                    out=wqall[:, g],
                    in_=wqh[:, 1024 * g:1024 * (g + 1)].rearrange(
                        "p (cc c) -> p cc c", cc=NCC),
                )
            def dma_xt_slice(j):
                nc.sync.dma_start(
                    out=xtall[:, :, 512 * j:512 * (j + 1)],
                    in_=xh[128 * j:128 * (j + 1), :].rearrange(
                        "p (cc t) -> p cc t", cc=NCC),
                )
            dma_wq_grp(0)             # q01
            dma_xt_slice(0)
            dma_wq_grp(1)             # k01
            dma_xt_slice(1)
            dma_wq_grp(2)             # q23
            dma_xt_slice(2)
            dma_wq_grp(3)             # k23
            dma_xt_slice(3)
            nc.sync.dma_start(
                out=wvall[:, :, :],
                in_=wvh[:, :].rearrange("p (cc c) -> p cc c", cc=NCC),
            )
            wpall = persist.tile([128, GQ // 128, C], mdt, tag="wpall", name="wpall")
            nc.sync.dma_start(
                out=wpall[:, :, :],
                in_=wph[:, :].rearrange("p (i c) -> p i c", i=GQ // 128),
            )
            wp = [wpall[:, i] for i in range(GQ // 128)]

            # ---- q/k projection (transposed out: [feat, t]) -------------
            # feature blocks in wqkvT cols: q:[0,256) k:[256,512) v:[512,768)
            # qk[0]=q01 cols [0,128), qk[1]=q23 [128,256), qk[2]=k01 [256,384), qk[3]=k23 [384,512)
            with tc.tile_pool(name="projps", bufs=4, space="PSUM") as pp:
                # group g: 0=q01, 1=k01, 2=q23, 3=k23 -> qk dst [0,2,1,3]
                for g, dst in [(0, 0), (1, 2), (2, 1), (3, 3)]:
                    for tqi in range(NTQ):
                        ps = pp.tile([128, 512], f32, tag="ps")
                        for cc in range(NCC):
                            nc.tensor.matmul(
                                ps[:, :],
                                wqall[:, g, cc, :],
                                xt[cc][:, 512 * tqi:512 * (tqi + 1)],
                                start=(cc == 0),
                                stop=(cc == NCC - 1 and not with_bias_qkv),
                            )
                        if with_bias_qkv:
                            col0 = [0, 256, 128, 384][g]
                            nc.tensor.matmul(
                                ps[:, :],
                                bqkv_r[:, col0:col0 + 128],
                                ones_r[:, 0:512],
                                start=False, stop=True,
                            )
                        nc.vector.tensor_copy(
                            out=qk[dst][:, 512 * tqi:512 * (tqi + 1)], in_=ps[:, :]
                        )

                # ---- v projection (natural out: [t, feat]) --------------
                for tb in range(NTK):
                    ps = pp.tile([128, GQ], f32, tag="psv")
                    for cc in range(NCC):
                        nc.tensor.matmul(
                            ps[:, :],
                            xt[cc][:, 128 * tb:128 * (tb + 1)],
                            wvall[:, cc, :],
                            start=(cc == 0),
                            stop=(cc == NCC - 1 and not with_bias_qkv),
                        )
                    if with_bias_qkv:
                        nc.tensor.matmul(
                            ps[:, :],
                            ones_r[:, 0:128],
                            bqkv_r[:, 512:768],
                            start=False, stop=True,
                        )
                    nc.vector.tensor_copy(
                        out=v1[:, tb, :, 0:64],
                        in_=ps[:].rearrange("p (h d) -> p h d", h=HLOC),
                    )

            # ---- attention + output projection, software-pipelined ------
            with (
                tc.tile_pool(name="spool", bufs=3, space="PSUM") as sp,
                tc.tile_pool(name="pvpool", bufs=3, space="PSUM") as pvp,
                tc.tile_pool(name="oppool", bufs=2, space="PSUM") as op,
                tc.tile_pool(name="epool", bufs=6) as ep,
                tc.tile_pool(name="rpool", bufs=4) as rp,
                tc.tile_pool(name="ostage", bufs=3) as ost,
            ):
                def emit_exp(tqi, h, tki):
                    # S^T block + exp -> returns the e tile (and its width)
                    tq0 = 512 * tqi
                    tk0 = 128 * tki
                    hp, hs = divmod(h, 2)
                    pb = 64 * hs
                    q_t, k_t = qk[hp], qk[2 + hp]
                    if tk0 < tq0:
                        s = sp.tile([128, 512], f32, tag="s")
                        nc.tensor.matmul(
                            s[:, :],
                            k_t[pb:pb + 64, tk0:tk0 + 128],
                            q_t[pb:pb + 64, tq0:tq0 + 512],
                            start=True, stop=True,
                        )
                        e = ep.tile([128, 512], mdt, tag="e")
                        nc.scalar.activation(out=e[:, :], in_=s[:, :],
                                             func=AF.Exp, scale=SCALE)
                        return e, 0
                    # diagonal-region block: visible tq cols [tk0, tq0+512)
                    m = (tk0 - tq0) // 128
                    w = 512 - 128 * m
                    s = sp.tile([128, 512], f32, tag="s")
                    nc.tensor.matmul(
                        s[:, 0:w],
                        k_t[pb:pb + 64, tk0:tk0 + 128],
                        q_t[pb:pb + 64, tq0 + 128 * m:tq0 + 512],
                        start=True, stop=True,
                    )
                    # causal mask on the first 128 visible cols
                    nc.vector.tensor_add(out=s[:, 0:128], in0=s[:, 0:128], in1=tri)
                    e = ep.tile([128, 512], mdt, tag="e")
                    nc.scalar.activation(out=e[:, 0:w], in_=s[:, 0:w],
                                         func=AF.Exp, scale=SCALE)
                    return e, m

                def emit_pv(pv, h, tki, e, m, first, last):
                    if m == 0:
                        nc.tensor.matmul(
                            pv[:, :], v1[:, tki, h, :], e[:, :],
                            start=first, stop=last,
                        )
                    else:
                        nc.tensor.matmul(
                            pv[:, 128 * m:512], v1[:, tki, h, :], e[:, 0:512 - 128 * m],
                            start=first, stop=last,
                        )

                def attention_head(tqi, h):
                    # S/exp runs two chunks ahead of PV so the tensor engine
                    # never waits on the scalar-engine exp
                    tq0 = 512 * tqi
                    n = tq0 // 128 + 4
                    pv = pvp.tile([65, 512], f32, tag="pv")
                    pend = []
                    for tki in range(n):
                        pend.append(emit_exp(tqi, h, tki) + (tki,))
                        if len(pend) > 2:
                            e, m, t0 = pend.pop(0)
                            emit_pv(pv, h, t0, e, m,
                                    first=(t0 == 0), last=(t0 == n - 1))
                    for e, m, t0 in pend:
                        emit_pv(pv, h, t0, e, m,
                                first=(t0 == 0), last=(t0 == n - 1))
                    # normalize: y = out / (denom + exp(sink))
                    hp, hs = divmod(h, 2)
                    pb = 64 * hs
                    r1a = rp.tile([1, 512], f32, tag="r1a")
                    nc.vector.tensor_scalar_add(
                        out=r1a, in0=pv[64:65, :], scalar1=sink_r[0:1, h:h + 1]
                    )
                    r1 = rp.tile([1, 512], f32, tag="r1")
                    nc.vector.reciprocal_approx_fast(out=r1, in_=r1a)
                    rb = rp.tile([64, 512], f32, tag="rb")
                    nc.gpsimd.partition_broadcast(rb, r1)
                    nc.vector.tensor_mul(
                        out=yT[hp][pb:pb + 64, tq0:tq0 + 512],
                        in0=pv[0:64, :],
                        in1=rb,
                    )

                def outproj_tb(tb):
                    stg = ost.tile([128, C], mdt, tag="ostg")
                    for co in range(2):
                        ps = op.tile([128, 512], f32, tag="ops")
                        for hd in range(2):
                            nc.tensor.matmul(
                                ps[:, :],
                                yT[hd][:, 128 * tb:128 * (tb + 1)],
                                wp[hd][:, 512 * co:512 * (co + 1)],
                                start=(hd == 0),
                                stop=(hd == 1 and not with_bias_proj),
                            )
                        if with_bias_proj:
                            nc.tensor.matmul(
                                ps[:, :],
                                ones_r[:, 0:128],
                                bp_r[:, 512 * co:512 * (co + 1)],
                                start=False, stop=True,
                            )
                        nc.vector.tensor_copy(out=stg[:, 512 * co:512 * (co + 1)], in_=ps[:, :])
                    nc.sync.dma_start(out=out[128 * tb:128 * (tb + 1), :], in_=stg[:, :])

                # outproj trails attention by one tq block, one tb emitted
                # after each head so outproj matmuls fill any exp-wait gaps
                for h in range(HLOC):
                    attention_head(0, h)
                for tqi in range(1, NTQ):
                    for h in range(HLOC):
                        attention_head(tqi, h)
                        outproj_tb(4 * (tqi - 1) + h)
                for h in range(HLOC):
                    outproj_tb(4 * (NTQ - 1) + h)

    nc.finalize()
    return nc


def make_core_inputs(x, W_qkv, b_qkv, W_proj, b_proj, sink_logit):
    """Host-side sharding: per-core input dicts (host does the transposes)."""
    import ml_dtypes
    bf16 = ml_dtypes.bfloat16

    x = np.asarray(x, dtype=np.float32)
    W_qkv = np.asarray(W_qkv, dtype=np.float32)
    b_qkv = np.asarray(b_qkv, dtype=np.float32)
    W_proj = np.asarray(W_proj, dtype=np.float32)
    b_proj = np.asarray(b_proj, dtype=np.float32)
    sink_logit = np.asarray(sink_logit, dtype=np.float32)

    # xh[j*128+p, cc*512+t'] = x[b][512j+t', 128cc+p]
    xhs = []
    for b in range(B):
        xT_b = x[b].T.reshape(NCC, 128, NTQ, 512)          # [cc, p, j, t']
        xh = xT_b.transpose(2, 1, 0, 3).reshape(NTQ * 128, NCC * 512)
        xhs.append(np.ascontiguousarray(xh).astype(bf16))

    in_maps = []
    for c in range(NCORES):
        b, g = divmod(c, 4)
        h0 = HLOC * g
        q_rows = slice(GQ * g, GQ * (g + 1))
        k_rows = slice(C + GQ * g, C + GQ * (g + 1))
        v_rows = slice(2 * C + GQ * g, 2 * C + GQ * (g + 1))
        w_slice = np.concatenate(
            [W_qkv[q_rows], W_qkv[k_rows], W_qkv[v_rows]], axis=0
        )  # (768, 1024)
        b_slice = np.concatenate(
            [b_qkv[q_rows], b_qkv[k_rows], b_qkv[v_rows]], axis=0
        )  # (768,)
        # A[p, cc, f] = w_slice[f, 128cc+p]
        A = w_slice.T.reshape(NCC, 128, F).transpose(1, 0, 2)  # [p, cc, f]
        grp_cols = [(0, 128), (256, 384), (128, 256), (384, 512)]
        wqh = np.concatenate([A[:, :, c0:c1] for c0, c1 in grp_cols],
                             axis=1).reshape(128, 4 * NCC * 128)
        wvh = A[:, :, 512:768].reshape(128, NCC * 256)
        wpT_s = W_proj[:, q_rows].T  # (256, 1024)
        wph = wpT_s.reshape(GQ // 128, 128, C).transpose(1, 0, 2).reshape(
            128, (GQ // 128) * C)
        sink = np.exp(sink_logit[h0:h0 + HLOC]).astype(np.float32)[None, :]
        in_maps.append({
            "xh": xhs[b],
            "wqh": np.ascontiguousarray(wqh).astype(bf16),
            "wvh": np.ascontiguousarray(wvh).astype(bf16),
            "bqkv": b_slice[None, :].astype(bf16),
            "wph": np.ascontiguousarray(wph).astype(bf16),
            "bp": (b_proj if g == 0 else np.zeros_like(b_proj))[None, :].astype(bf16),
            "sink4": sink,
        })
    return in_maps


_NC_CACHE = {}


def kernel(x, W_qkv, b_qkv, W_proj, b_proj, sink_logit, _trace=False):
    from concourse.bass_utils import run_bass_kernel_spmd  # noqa: F401 (path set below)

    in_maps = make_core_inputs(x, W_qkv, b_qkv, W_proj, b_proj, sink_logit)
    with_bias_qkv = bool(np.any(np.asarray(b_qkv)))
    with_bias_proj = bool(np.any(np.asarray(b_proj)))
    key = ("bfloat16", with_bias_qkv, with_bias_proj)
    if key not in _NC_CACHE:
        _NC_CACHE[key] = build_nc("bfloat16", with_bias_qkv, with_bias_proj)
    nc = _NC_CACHE[key]

    from concourse.bass_utils import run_bass_kernel_spmd
    res = run_bass_kernel_spmd(nc, in_maps, core_ids=list(range(NCORES)), trace=_trace)

    outs = [np.asarray(res.results[c]["out"], dtype=np.float32) for c in range(NCORES)]
    y = np.empty((B, T, C), dtype=np.float32)
    for b in range(B):
        y[b] = outs[4 * b] + outs[4 * b + 1] + outs[4 * b + 2] + outs[4 * b + 3]
    if _trace:
        return y, res
    return y


# make bass importable at module load so `from kernel import kernel` works
_import_bass()
